# revision 3
# baseline (speedup 1.0000x reference)
"""Multi-head attention Trainium2 kernel (8 NeuronCores, SPMD).

Global software pipeline: stage n sweeps scores+exp for pair n while running
pair n-1's PV as four sequential per-ic passes (16 accumulating matmuls
each, closed with stop before the next ic starts) ping-ponging between two
PSUM banks -- PSUM start=True zeroes a whole 2KB region, so concurrent
per-ic groups in one bank are illegal.  Norm is per-ic, inline, right after
each pass closes.

Layout decisions: host-transposed fp16 inputs/weights, Q^T/K^T f32r via
projection, V[t, o] fp16 with mask e^mb folded into V rows, flipped PV
(lhsT = et chunk, rhs = V with ones column -> Z on the free dim), exp split
ACT (Exp, fused scale) / Pool (pow base e^0.125 fed by DVE PSUM->SBUF copy),
x^T via DMA transpose (PE transpose on the latency-critical tail super),
out projection accumulates 2 hd-chunks of 128.
"""
import sys

sys.path.insert(0, "/opt/trn_rl_repo")

import numpy as np

import concourse.bass as bass
import concourse.tile as tile
from concourse import bacc, mybir
from concourse.bass_utils import run_bass_kernel_spmd

P = 128
T = 2048
F = 512
OB = 256
NH = 4
DK = 64
FO = F // P
NT = T // P
JT = NT
ISUP = 512
NSU = T // ISUP
NCH = ISUP // P
CW = 512
NC4 = T // CW
EPS = 1e-8
NPAIR = NSU * NH

f32 = mybir.dt.float32
f32r = mybir.dt.float32r
fp16 = mybir.dt.float16

EXP_BASE = float(np.exp(0.125))
POOL_T = {0, 2, 4, 7, 9, 11, 14}    # within-pair chunk idx -> Pool pow

_CACHE = {}


def _build():
    nc = bacc.Bacc("TRN2", target_bir_lowering=False, debug=False, num_devices=8)

    xqT = nc.dram_tensor("xqT", (NC4, P, FO, CW), fp16, kind="ExternalInput").ap()
    xkT = nc.dram_tensor("xkT", (NC4, P, FO, CW), fp16, kind="ExternalInput").ap()
    xvT = nc.dram_tensor("xvT", (NC4, P, FO, CW), fp16, kind="ExternalInput").ap()
    wqT = nc.dram_tensor("wqT", (P, FO, OB), fp16, kind="ExternalInput").ap()
    wkT = nc.dram_tensor("wkT", (P, FO, OB), fp16, kind="ExternalInput").ap()
    wvT = nc.dram_tensor("wvT", (P, FO, OB), fp16, kind="ExternalInput").ap()
    woT = nc.dram_tensor("woT", (P, 2, F), fp16, kind="ExternalInput").ap()
    bqr = nc.dram_tensor("bqr", (P, OB // P), f32, kind="ExternalInput").ap()
    bkr = nc.dram_tensor("bkr", (P, OB // P), f32, kind="ExternalInput").ap()
    bvb = nc.dram_tensor("bvb", (P, OB), f32, kind="ExternalInput").ap()
    expmb = nc.dram_tensor("expmb", (P, NT), f32, kind="ExternalInput").ap()
    ident = nc.dram_tensor("ident", (P, P), fp16, kind="ExternalInput").ap()
    out = nc.dram_tensor("out", (T, F), fp16, kind="ExternalOutput").ap()

    with tile.TileContext(nc) as tc:
        with tc.tile_pool(name="const", bufs=1) as cpool, \
             tc.tile_pool(name="xs", bufs=2) as xpool, \
             tc.tile_pool(name="act", bufs=1) as apool, \
             tc.tile_pool(name="et", bufs=34) as epool, \
             tc.tile_pool(name="sc", bufs=8) as scpool, \
             tc.tile_pool(name="nrm", bufs=4) as npool, \
             tc.tile_pool(name="xout", bufs=2) as xopool, \
             tc.tile_pool(name="os", bufs=3) as ospool, \
             tc.tile_pool(name="ps", bufs=1, space="PSUM") as ps:

            ebase = cpool.tile([P, ISUP], f32, tag="ebase")
            nc.gpsimd.memset(ebase[:], EXP_BASE)

            # ---- DMA emission order = transfer order
            xq_sb = [None] * NC4
            xk_sb = [None] * NC4
            xv_sb = [None] * NC4

            def load_chunk(store, xdram, name, c):
                t = xpool.tile([P, FO, CW], fp16, tag=f"xs_{name}",
                               name=f"xs_{name}_{c}")
                nc.sync.dma_start(out=t[:], in_=xdram[c])
                store[c] = t

            def wtile(shape, dt_, tag, src):
                w = cpool.tile(shape, dt_, tag=tag, name=tag)
                nc.sync.dma_start(out=w[:], in_=src[:])
                return w

            load_chunk(xq_sb, xqT, "q", 0)
            wq_sb = cpool.tile([P, FO, OB], fp16, tag="wq", name="wq")
            nc.scalar.dma_start(out=wq_sb[:], in_=wqT[:])
            bq_sb = cpool.tile([P, OB // P], f32, tag="bq", name="bq")
            nc.scalar.dma_start(out=bq_sb[:], in_=bqr[:])
            xk0 = xpool.tile([P, FO, CW], fp16, tag="xs_k", name="xs_k_0")
            nc.gpsimd.dma_start(out=xk0[:], in_=xkT[0])
            xk_sb[0] = xk0
            wk_sb = wtile([P, FO, OB], fp16, "wk", wkT)
            bk_sb = wtile([P, OB // P], f32, "bk", bkr)
            load_chunk(xk_sb, xkT, "k", 1)
            load_chunk(xk_sb, xkT, "k", 2)
            load_chunk(xk_sb, xkT, "k", 3)
            wv_sb = wtile([P, FO, OB], fp16, "wv", wvT)
            bv_sb = wtile([P, OB], f32, "bv", bvb)
            mb_sb = wtile([P, NT], f32, "mb", expmb)
            load_chunk(xv_sb, xvT, "v", 0)
            load_chunk(xv_sb, xvT, "v", 1)
            load_chunk(xv_sb, xvT, "v", 2)
            load_chunk(xv_sb, xvT, "v", 3)
            load_chunk(xq_sb, xqT, "q", 1)
            wo_sb = wtile([P, 2, F], fp16, "wo", woT)
            id_sb = wtile([P, P], fp16, "ident", ident)
            load_chunk(xq_sb, xqT, "q", 2)
            load_chunk(xq_sb, xqT, "q", 3)

            # ---- persistent activations ----
            QT = apool.tile([P, OB // P, T], f32r, tag="QT")
            KT = apool.tile([P, OB // P, T], f32r, tag="KT")
            V2 = apool.tile([P, NT, NH, DK + 1], fp16, tag="V2")
            nc.vector.memset(V2[:, :, :, DK:DK + 1], 1.0)

            # ---- emitters ----
            def emit_qk_proj(w_sb, bias_sb, dst, xs, c):
                for po in range(OB // P):
                    pp = ps.tile([P, CW], f32, tag="pp", bufs=2,
                                 name=f"pp_{c}_{po}")
                    for fo in range(FO):
                        nc.tensor.matmul(
                            pp[:],
                            w_sb[:, fo, po * P:(po + 1) * P],
                            xs[:, fo, :],
                            start=(fo == 0),
                            stop=(fo == FO - 1),
                        )
                    nc.vector.tensor_scalar_add(
                        dst[:, po, c * CW:(c + 1) * CW],
                        pp[:],
                        bias_sb[:, po:po + 1],
                    )

            def emit_v_proj(t):
                xs = xv_sb[t // (CW // P)]
                tt = t % (CW // P)
                pp = ps.tile([P, OB], f32, tag="pp", bufs=2, name=f"ppv_{t}")
                for fo in range(FO):
                    nc.tensor.matmul(
                        pp[:],
                        xs[:, fo, tt * P:(tt + 1) * P],
                        wv_sb[:, fo, :],
                        start=(fo == 0),
                        stop=(fo == FO - 1),
                    )
                nc.vector.tensor_add(
                    V2[:, t, :, 0:DK],
                    pp[:].rearrange("p (h d) -> p h d", h=NH),
                    bv_sb[:].rearrange("p (h d) -> p h d", h=NH),
                )
                nc.gpsimd.tensor_scalar(
                    V2[:, t, :, :],
                    V2[:, t, :, :],
                    mb_sb[:, t:t + 1],
                    None,
                    mybir.AluOpType.mult,
                )

            def emit_exp(st, nm, t, pool_ok=True):
                et = epool.tile([P, ISUP], fp16, tag="et", name=f"et_{nm}")
                if pool_ok and t in POOL_T:
                    stc = scpool.tile([P, ISUP], f32, tag="sc", name=f"sc_{nm}")
                    nc.vector.tensor_copy(stc[:], st[:])
                    nc.gpsimd.tensor_tensor(
                        et[:], ebase[:], stc[:], mybir.AluOpType.pow)
                else:
                    nc.scalar.activation(
                        et[:], st[:], mybir.ActivationFunctionType.Exp,
                        scale=0.125)
                return et

            def emit_scores(pi, jt):
                su, h = pi // NH, pi % NH
                qoff = (h % 2) * DK
                po = h // 2
                st = ps.tile([P, ISUP], f32, tag="st", bufs=4,
                             name=f"st_{pi}_{jt}")
                nc.tensor.matmul(
                    st[:],
                    KT[qoff:qoff + DK, po, jt * P:(jt + 1) * P],
                    QT[qoff:qoff + DK, po, su * ISUP:(su + 1) * ISUP],
                    start=True,
                    stop=True,
                )
                return st

            def emit_pv_chunk(rec, ic, phase):
                # 4 of the 16 accumulating matmuls of pass ic (bank ic%2)
                xp, h = rec["xp"], rec["h"]
                for j in range(4):
                    jt = 4 * phase + j
                    nc.tensor.matmul(
                        xp[:, ic % 2, 0:DK + 1],
                        rec["ets"][jt][:, ic * P:(ic + 1) * P],
                        V2[:, jt, h, :],
                        start=(jt == 0),
                        stop=(jt == JT - 1),
                    )

            def emit_norm_ic(rec, ic):
                # pass ic just closed: r = 1/(Z+eps); x_sb[ic, h] = x * r
                h, xp, x_sb = rec["h"], rec["xp"], rec["x_sb"]
                zr = npool.tile([P, 1], f32, tag="zr",
                                name=f"zr_{rec['pi']}_{ic}")
                nc.vector.tensor_scalar_add(
                    zr[:], xp[:, ic % 2, DK:DK + 1], EPS)
                rr = npool.tile([P, 1], f32, tag="rr",
                                name=f"rr_{rec['pi']}_{ic}")
                nc.vector.reciprocal(rr[:], zr[:])
                nc.vector.tensor_scalar(
                    x_sb[:, ic, h, :],
                    xp[:, ic % 2, 0:DK],
                    rr[:, 0:1],
                    None,
                    mybir.AluOpType.mult,
                )

            def emit_xt_dma(x_sb, xt, ic):
                nc.sync.dma_start_transpose(
                    xt[:, :, ic * P:(ic + 1) * P],
                    x_sb[:, ic, :, :],
                )

            def emit_xt_pe(x_sb, xt, ic):
                tr = ps.tile([P, 2, P], fp16, tag="pp", bufs=2,
                             name=f"tr_{ic}")
                for ch in range(2):
                    nc.tensor.transpose(
                        tr[:, ch, :],
                        x_sb[:, ic, 2 * ch:2 * ch + 2, :],
                        id_sb[:],
                    )
                nc.vector.tensor_copy(xt[:, :, ic * P:(ic + 1) * P], tr[:])

            def emit_outproj(xt, su, ic, os_on_act=False):
                pp = ps.tile([P, F], f32, tag="pp", bufs=2,
                             name=f"op_{su}_{ic}")
                for ch in range(2):
                    nc.tensor.matmul(
                        pp[:],
                        xt[:, ch, ic * P:(ic + 1) * P],
                        wo_sb[:, ch, :],
                        start=(ch == 0),
                        stop=(ch == 1),
                    )
                os = ospool.tile([P, F], fp16, tag="os", name=f"os_{su}_{ic}")
                if os_on_act:
                    nc.scalar.copy(os[:], pp[:])
                else:
                    nc.vector.tensor_copy(os[:], pp[:])
                nc.sync.dma_start(
                    out=out[(su * NCH + ic) * P:(su * NCH + ic + 1) * P, :],
                    in_=os[:],
                )

            # ---- schedule ----
            recs = []
            x_sbs = [None] * NSU
            xts = [None] * NSU

            emit_qk_proj(wq_sb, bq_sb, QT, xq_sb[0], 0)
            emit_qk_proj(wk_sb, bk_sb, KT, xk_sb[0], 0)

            def dance(n, t):
                if n == 0:
                    if t == 2:
                        emit_qk_proj(wk_sb, bk_sb, KT, xk_sb[1], 1)
                    elif t == 5:
                        emit_qk_proj(wk_sb, bk_sb, KT, xk_sb[2], 2)
                    elif t == 7:
                        emit_qk_proj(wk_sb, bk_sb, KT, xk_sb[3], 3)
                    elif t >= 8:
                        emit_v_proj(2 * (t - 8))
                        emit_v_proj(2 * (t - 8) + 1)
                    return
                if t == 8 and n in (3, 5, 9):
                    c = {3: 1, 5: 2, 9: 3}[n]
                    emit_qk_proj(wq_sb, bq_sb, QT, xq_sb[c], c)
                if n in (5, 9, 13):
                    su = (n - 5) // 4
                    if t in (1, 3, 5, 7):
                        emit_xt_dma(x_sbs[su], xts[su], (t - 1) // 2)
                    elif t in (9, 11, 13, 15):
                        emit_outproj(xts[su], su, (t - 9) // 2, os_on_act=True)
                if n == 16:
                    # su3 tail work interleaved with pair15's PV passes
                    su = NSU - 1
                    if t in (5, 9, 13):
                        emit_xt_pe(x_sbs[su], xts[su], (t - 5) // 4)
                    elif t in (7, 11, 15):
                        emit_outproj(xts[su], su, (t - 7) // 4, os_on_act=True)

            for n in range(NPAIR + 1):
                if n < NPAIR:
                    su, h = n // NH, n % NH
                    if h == 0:
                        x_sbs[su] = xopool.tile(
                            [P, NCH, NH, DK], fp16, tag="xsb",
                            name=f"xsb_{su}")
                        xts[su] = xopool.tile(
                            [P, 2, ISUP], fp16, tag="xt", name=f"xt_{su}")
                    rec = {
                        "pi": n, "h": h, "su": su,
                        "xp": ps.tile([P, 2, CW], f32, tag="xp", bufs=1,
                                      name=f"xp_{n}"),
                        "ets": [],
                        "x_sb": x_sbs[su],
                    }
                    recs.append(rec)
                    st_prev = emit_scores(n, 0)
                for t in range(JT):
                    if n < NPAIR:
                        recs[n]["ets"].append(
                            emit_exp(st_prev, f"{n}_{t}", t,
                                     pool_ok=(n, t) != (15, 15)))
                        if t + 1 < JT:
                            st_prev = emit_scores(n, t + 1)
                    if n >= 1:
                        prev = recs[n - 1]
                        ic, phase = t // 4, t % 4
                        emit_pv_chunk(prev, ic, phase)
                        if phase == 3:
                            emit_norm_ic(prev, ic)
                            if ic == NCH - 1:
                                prev["ets"] = None
                    dance(n, t)

            # ---- tail: last outproj (ic3 of su3) ----
            su = NSU - 1
            emit_xt_pe(x_sbs[su], xts[su], 3)
            emit_outproj(xts[su], su, 3, os_on_act=True)

    nc.compile()
    return nc


def _prep_in_maps(query, key, value, mask, Wq, bq, Wk, bk, Wv, bv, Wo):
    in_maps = []
    for c in range(8):
        b = c // 2
        hh = c % 2
        ob = slice(hh * OB, (hh + 1) * OB)

        def xprep(x):
            xt = np.ascontiguousarray(np.asarray(x).T)     # [F, T]
            return np.ascontiguousarray(
                xt.reshape(FO, P, NC4, CW).transpose(2, 1, 0, 3)
            ).astype(np.float16)

        def wprep(W):
            wt = np.ascontiguousarray(np.asarray(W)[ob, :].T)   # [F, OB]
            return np.ascontiguousarray(
                wt.reshape(FO, P, OB).transpose(1, 0, 2)).astype(np.float16)

        woTh = np.ascontiguousarray(np.asarray(Wo)[:, ob].T)    # [OB, F]
        woTh = np.ascontiguousarray(
            woTh.reshape(2, P, F).transpose(1, 0, 2)).astype(np.float16)

        emb = (np.asarray(mask)[b, 0, :] != 0).astype(np.float32)
        emb = np.ascontiguousarray(emb.reshape(NT, P).T)

        in_maps.append({
            "xqT": xprep(query[b]),
            "xkT": xprep(key[b]),
            "xvT": xprep(value[b]),
            "wqT": wprep(Wq),
            "wkT": wprep(Wk),
            "wvT": wprep(Wv),
            "woT": woTh,
            "bqr": np.ascontiguousarray(bq[ob].reshape(OB // P, P).T),
            "bkr": np.ascontiguousarray(bk[ob].reshape(OB // P, P).T),
            "bvb": np.ascontiguousarray(np.tile(bv[ob][None, :], (P, 1))),
            "expmb": emb,
            "ident": np.eye(P, dtype=np.float16),
        })
    return in_maps


def kernel(query, key, value, mask, Wq, bq, Wk, bk, Wv, bv, Wo, bo):
    query = np.asarray(query, dtype=np.float32)
    key = np.asarray(key, dtype=np.float32)
    value = np.asarray(value, dtype=np.float32)
    mask = np.asarray(mask)
    Wq = np.asarray(Wq, dtype=np.float32)
    bq = np.asarray(bq, dtype=np.float32)
    Wk = np.asarray(Wk, dtype=np.float32)
    bk = np.asarray(bk, dtype=np.float32)
    Wv = np.asarray(Wv, dtype=np.float32)
    bv = np.asarray(bv, dtype=np.float32)
    Wo = np.asarray(Wo, dtype=np.float32)
    bo = np.asarray(bo, dtype=np.float32)

    if "nc" not in _CACHE:
        _CACHE["nc"] = _build()
    nc = _CACHE["nc"]

    B = query.shape[0]
    in_maps = _prep_in_maps(query, key, value, mask, Wq, bq, Wk, bk, Wv, bv, Wo)
    res = run_bass_kernel_spmd(nc, in_maps, core_ids=list(range(8)))

    outv = np.empty((B, T, F), dtype=np.float32)
    for b in range(B):
        outv[b] = (res.results[2 * b]["out"].astype(np.float32)
                   + res.results[2 * b + 1]["out"].astype(np.float32)
                   + bo[None, :])
    return outv


# revision 5
# speedup vs baseline: 1.0453x; 1.0453x over previous
"""Multi-head attention Trainium2 kernel (8 NeuronCores, SPMD).

Global software pipeline: stage n sweeps scores+exp for pair n while running
pair n-1's PV as four sequential per-ic passes (16 accumulating matmuls
each, closed with stop before the next ic starts) ping-ponging between two
PSUM banks -- PSUM start=True zeroes a whole 2KB region, so concurrent
per-ic groups in one bank are illegal.  Norm is per-ic, inline, right after
each pass closes.

Layout decisions: host-transposed fp16 inputs/weights, Q^T/K^T f32r via
projection, V[t, o] fp16 with mask e^mb folded into V rows, flipped PV
(lhsT = et chunk, rhs = V with ones column -> Z on the free dim), exp split
ACT (Exp, fused scale) / Pool (pow base e^0.125 fed by DVE PSUM->SBUF copy),
x^T via DMA transpose (PE transpose on the latency-critical tail super),
out projection accumulates 2 hd-chunks of 128.
"""
import sys

sys.path.insert(0, "/opt/trn_rl_repo")

import numpy as np

import concourse.bass as bass
import concourse.tile as tile
from concourse import bacc, mybir
from concourse.bass_utils import run_bass_kernel_spmd

P = 128
T = 2048
F = 512
OB = 256
NH = 4
DK = 64
FO = F // P
NT = T // P
JT = NT
ISUP = 512
NSU = T // ISUP
NCH = ISUP // P
CW = 512
NC4 = T // CW
EPS = 1e-8
NPAIR = NSU * NH

f32 = mybir.dt.float32
f32r = mybir.dt.float32r
fp16 = mybir.dt.float16

EXP_BASE = float(np.exp(0.125))
POOL_T = {0, 2, 4, 7, 9, 11, 14}    # within-pair chunk idx -> Pool pow

_CACHE = {}


def _build():
    nc = bacc.Bacc("TRN2", target_bir_lowering=False, debug=False, num_devices=8)

    xqT = nc.dram_tensor("xqT", (NC4, P, FO, CW), fp16, kind="ExternalInput").ap()
    xkT = nc.dram_tensor("xkT", (NC4, P, FO, CW), fp16, kind="ExternalInput").ap()
    xvT = nc.dram_tensor("xvT", (NC4, P, FO, CW), fp16, kind="ExternalInput").ap()
    wqT = nc.dram_tensor("wqT", (P, FO, OB), fp16, kind="ExternalInput").ap()
    wkT = nc.dram_tensor("wkT", (P, FO, OB), fp16, kind="ExternalInput").ap()
    wvT = nc.dram_tensor("wvT", (P, FO, OB), fp16, kind="ExternalInput").ap()
    woT = nc.dram_tensor("woT", (P, 2, F), fp16, kind="ExternalInput").ap()
    bqr = nc.dram_tensor("bqr", (P, OB // P), f32, kind="ExternalInput").ap()
    bkr = nc.dram_tensor("bkr", (P, OB // P), f32, kind="ExternalInput").ap()
    bvb = nc.dram_tensor("bvb", (P, OB), f32, kind="ExternalInput").ap()
    expmb = nc.dram_tensor("expmb", (P, NT), f32, kind="ExternalInput").ap()
    ident = nc.dram_tensor("ident", (P, P), fp16, kind="ExternalInput").ap()
    out = nc.dram_tensor("out", (T, F), fp16, kind="ExternalOutput").ap()

    with tile.TileContext(nc) as tc:
        with tc.tile_pool(name="const", bufs=1) as cpool, \
             tc.tile_pool(name="xs", bufs=2) as xpool, \
             tc.tile_pool(name="act", bufs=1) as apool, \
             tc.tile_pool(name="et", bufs=34) as epool, \
             tc.tile_pool(name="sc", bufs=8) as scpool, \
             tc.tile_pool(name="nrm", bufs=4) as npool, \
             tc.tile_pool(name="xout", bufs=2) as xopool, \
             tc.tile_pool(name="os", bufs=3) as ospool, \
             tc.tile_pool(name="ps", bufs=1, space="PSUM") as ps:

            ebase = cpool.tile([P, ISUP], f32, tag="ebase")
            nc.gpsimd.memset(ebase[:], EXP_BASE)

            # ---- DMA emission order = transfer order
            xq_sb = [None] * NC4
            xk_sb = [None] * NC4
            xv_sb = [None] * NC4

            def load_chunk(store, xdram, name, c):
                t = xpool.tile([P, FO, CW], fp16, tag=f"xs_{name}",
                               name=f"xs_{name}_{c}")
                nc.sync.dma_start(out=t[:], in_=xdram[c])
                store[c] = t

            def wtile(shape, dt_, tag, src):
                w = cpool.tile(shape, dt_, tag=tag, name=tag)
                nc.sync.dma_start(out=w[:], in_=src[:])
                return w

            load_chunk(xq_sb, xqT, "q", 0)
            wq_sb = cpool.tile([P, FO, OB], fp16, tag="wq", name="wq")
            nc.scalar.dma_start(out=wq_sb[:], in_=wqT[:])
            bq_sb = cpool.tile([P, OB // P], f32, tag="bq", name="bq")
            nc.scalar.dma_start(out=bq_sb[:], in_=bqr[:])
            xk0 = xpool.tile([P, FO, CW], fp16, tag="xs_k", name="xs_k_0")
            nc.gpsimd.dma_start(out=xk0[:], in_=xkT[0])
            xk_sb[0] = xk0
            wk_sb = wtile([P, FO, OB], fp16, "wk", wkT)
            bk_sb = wtile([P, OB // P], f32, "bk", bkr)
            load_chunk(xk_sb, xkT, "k", 1)
            load_chunk(xk_sb, xkT, "k", 2)
            load_chunk(xk_sb, xkT, "k", 3)
            wv_sb = wtile([P, FO, OB], fp16, "wv", wvT)
            bv_sb = wtile([P, OB], f32, "bv", bvb)
            mb_sb = wtile([P, NT], f32, "mb", expmb)
            load_chunk(xv_sb, xvT, "v", 0)
            load_chunk(xv_sb, xvT, "v", 1)
            load_chunk(xv_sb, xvT, "v", 2)
            load_chunk(xv_sb, xvT, "v", 3)
            load_chunk(xq_sb, xqT, "q", 1)
            wo_sb = wtile([P, 2, F], fp16, "wo", woT)
            id_sb = wtile([P, P], fp16, "ident", ident)
            load_chunk(xq_sb, xqT, "q", 2)
            load_chunk(xq_sb, xqT, "q", 3)

            # ---- PE p-state warm-up on dummy data ----
            warm = cpool.tile([P, P], fp16, tag="warm")
            nc.vector.memset(warm[:], 0.0)
            for w in range(40):
                wp = ps.tile([P, ISUP], f32, tag="st", bufs=4,
                             name=f"warm_{w}")
                nc.tensor.matmul(wp[:, 0:P], warm[:], warm[:],
                                 start=True, stop=True)

            # ---- persistent activations ----
            QT = apool.tile([P, OB // P, T], f32r, tag="QT")
            KT = apool.tile([P, OB // P, T], f32r, tag="KT")
            V2 = apool.tile([P, NT, NH, DK + 1], fp16, tag="V2")
            nc.vector.memset(V2[:, :, :, DK:DK + 1], 1.0)

            # ---- emitters ----
            def emit_qk_proj(w_sb, bias_sb, dst, xs, c):
                for po in range(OB // P):
                    pp = ps.tile([P, CW], f32, tag="pp", bufs=2,
                                 name=f"pp_{c}_{po}")
                    for fo in range(FO):
                        nc.tensor.matmul(
                            pp[:],
                            w_sb[:, fo, po * P:(po + 1) * P],
                            xs[:, fo, :],
                            start=(fo == 0),
                            stop=(fo == FO - 1),
                        )
                    nc.vector.tensor_scalar_add(
                        dst[:, po, c * CW:(c + 1) * CW],
                        pp[:],
                        bias_sb[:, po:po + 1],
                    )

            def emit_v_proj(t):
                xs = xv_sb[t // (CW // P)]
                tt = t % (CW // P)
                pp = ps.tile([P, OB], f32, tag="pp", bufs=2, name=f"ppv_{t}")
                for fo in range(FO):
                    nc.tensor.matmul(
                        pp[:],
                        xs[:, fo, tt * P:(tt + 1) * P],
                        wv_sb[:, fo, :],
                        start=(fo == 0),
                        stop=(fo == FO - 1),
                    )
                nc.vector.tensor_add(
                    V2[:, t, :, 0:DK],
                    pp[:].rearrange("p (h d) -> p h d", h=NH),
                    bv_sb[:].rearrange("p (h d) -> p h d", h=NH),
                )
                nc.scalar.activation(
                    V2[:, t, :, :],
                    V2[:, t, :, :],
                    mybir.ActivationFunctionType.Copy,
                    scale=mb_sb[:, t:t + 1],
                )

            def emit_exp(st, nm, t, pool_ok=True):
                et = epool.tile([P, ISUP], fp16, tag="et", name=f"et_{nm}")
                if pool_ok and t in POOL_T:
                    stc = scpool.tile([P, ISUP], f32, tag="sc", name=f"sc_{nm}")
                    nc.vector.tensor_copy(stc[:], st[:])
                    nc.gpsimd.tensor_tensor(
                        et[:], ebase[:], stc[:], mybir.AluOpType.pow)
                else:
                    nc.scalar.activation(
                        et[:], st[:], mybir.ActivationFunctionType.Exp,
                        scale=0.125)
                return et

            def emit_scores(pi, jt):
                su, h = pi // NH, pi % NH
                qoff = (h % 2) * DK
                po = h // 2
                st = ps.tile([P, ISUP], f32, tag="st", bufs=4,
                             name=f"st_{pi}_{jt}")
                nc.tensor.matmul(
                    st[:],
                    KT[qoff:qoff + DK, po, jt * P:(jt + 1) * P],
                    QT[qoff:qoff + DK, po, su * ISUP:(su + 1) * ISUP],
                    start=True,
                    stop=True,
                )
                return st

            def emit_pv_chunk(rec, ic, phase):
                # 4 of the 16 accumulating matmuls of pass ic (bank ic%2)
                xp, h = rec["xp"], rec["h"]
                for j in range(4):
                    jt = 4 * phase + j
                    nc.tensor.matmul(
                        xp[:, ic % 2, 0:DK + 1],
                        rec["ets"][jt][:, ic * P:(ic + 1) * P],
                        V2[:, jt, h, :],
                        start=(jt == 0),
                        stop=(jt == JT - 1),
                    )

            def emit_norm_ic(rec, ic):
                # pass ic just closed: r = 1/(Z+eps); x_sb[ic, h] = x * r
                h, xp, x_sb = rec["h"], rec["xp"], rec["x_sb"]
                zr = npool.tile([P, 1], f32, tag="zr",
                                name=f"zr_{rec['pi']}_{ic}")
                nc.vector.tensor_scalar_add(
                    zr[:], xp[:, ic % 2, DK:DK + 1], EPS)
                rr = npool.tile([P, 1], f32, tag="rr",
                                name=f"rr_{rec['pi']}_{ic}")
                nc.vector.reciprocal(rr[:], zr[:])
                nc.vector.tensor_scalar(
                    x_sb[:, ic, h, :],
                    xp[:, ic % 2, 0:DK],
                    rr[:, 0:1],
                    None,
                    mybir.AluOpType.mult,
                )

            def emit_xt_dma(x_sb, xt, ic):
                nc.sync.dma_start_transpose(
                    xt[:, :, ic * P:(ic + 1) * P],
                    x_sb[:, ic, :, :],
                )

            def emit_xt_pe(x_sb, xt, ic):
                tr = ps.tile([P, 2, P], fp16, tag="pp", bufs=2,
                             name=f"tr_{ic}")
                for ch in range(2):
                    nc.tensor.transpose(
                        tr[:, ch, :],
                        x_sb[:, ic, 2 * ch:2 * ch + 2, :],
                        id_sb[:],
                    )
                nc.vector.tensor_copy(xt[:, :, ic * P:(ic + 1) * P], tr[:])

            def emit_outproj(xt, su, ic, os_on_act=False):
                pp = ps.tile([P, F], f32, tag="pp", bufs=2,
                             name=f"op_{su}_{ic}")
                for ch in range(2):
                    nc.tensor.matmul(
                        pp[:],
                        xt[:, ch, ic * P:(ic + 1) * P],
                        wo_sb[:, ch, :],
                        start=(ch == 0),
                        stop=(ch == 1),
                    )
                os = ospool.tile([P, F], fp16, tag="os", name=f"os_{su}_{ic}")
                if os_on_act:
                    nc.scalar.copy(os[:], pp[:])
                else:
                    nc.vector.tensor_copy(os[:], pp[:])
                nc.sync.dma_start(
                    out=out[(su * NCH + ic) * P:(su * NCH + ic + 1) * P, :],
                    in_=os[:],
                )

            # ---- schedule ----
            recs = []
            x_sbs = [None] * NSU
            xts = [None] * NSU

            emit_qk_proj(wq_sb, bq_sb, QT, xq_sb[0], 0)
            emit_qk_proj(wk_sb, bk_sb, KT, xk_sb[0], 0)

            def dance(n, t):
                if n == 0:
                    if t == 2:
                        emit_qk_proj(wk_sb, bk_sb, KT, xk_sb[1], 1)
                    elif t == 5:
                        emit_qk_proj(wk_sb, bk_sb, KT, xk_sb[2], 2)
                    elif t == 7:
                        emit_qk_proj(wk_sb, bk_sb, KT, xk_sb[3], 3)
                    elif 8 <= t < 14:
                        emit_v_proj(2 * (t - 8))
                        emit_v_proj(2 * (t - 8) + 1)
                    return
                if n == 1 and t < 4:
                    emit_v_proj(12 + t)
                if t == 8 and n in (3, 5, 9):
                    c = {3: 1, 5: 2, 9: 3}[n]
                    emit_qk_proj(wq_sb, bq_sb, QT, xq_sb[c], c)
                if n in (5, 9, 13):
                    su = (n - 5) // 4
                    if t in (1, 3, 5, 7):
                        emit_xt_dma(x_sbs[su], xts[su], (t - 1) // 2)
                    elif t in (9, 11, 13, 15):
                        emit_outproj(xts[su], su, (t - 9) // 2, os_on_act=True)
                if n == 16:
                    # su3 tail work interleaved with pair15's PV passes
                    su = NSU - 1
                    if t in (5, 9, 13):
                        emit_xt_pe(x_sbs[su], xts[su], (t - 5) // 4)
                    elif t in (7, 11, 15):
                        emit_outproj(xts[su], su, (t - 7) // 4, os_on_act=True)

            for n in range(NPAIR + 1):
                if n < NPAIR:
                    su, h = n // NH, n % NH
                    if h == 0:
                        x_sbs[su] = xopool.tile(
                            [P, NCH, NH, DK], fp16, tag="xsb",
                            name=f"xsb_{su}")
                        xts[su] = xopool.tile(
                            [P, 2, ISUP], fp16, tag="xt", name=f"xt_{su}")
                    rec = {
                        "pi": n, "h": h, "su": su,
                        "xp": ps.tile([P, 2, CW], f32, tag="xp", bufs=1,
                                      name=f"xp_{n}"),
                        "ets": [],
                        "x_sb": x_sbs[su],
                    }
                    recs.append(rec)
                    st_prev = emit_scores(n, 0)
                for t in range(JT):
                    if n < NPAIR:
                        recs[n]["ets"].append(
                            emit_exp(st_prev, f"{n}_{t}", t,
                                     pool_ok=(n, t) != (15, 15)))
                        if t + 1 < JT:
                            st_prev = emit_scores(n, t + 1)
                    if n == 1:
                        prev = recs[0]
                        if t < 8:
                            emit_pv_chunk(prev, t % 2, t // 2)
                        elif t == 8:
                            emit_norm_ic(prev, 0)
                        elif t == 9:
                            emit_pv_chunk(prev, 2, 0)
                            emit_norm_ic(prev, 1)
                        elif t < 15:
                            ic = 3 if t % 2 == 0 else 2
                            emit_pv_chunk(prev, ic, (t - 9) // 2)
                        else:
                            emit_pv_chunk(prev, 2, 3)
                    elif n >= 2:
                        if n == 2 and t == 0:
                            emit_pv_chunk(recs[0], 3, 3)
                            emit_norm_ic(recs[0], 2)
                            emit_norm_ic(recs[0], 3)
                            recs[0]["ets"] = None
                        prev = recs[n - 1]
                        ic, phase = t // 4, t % 4
                        emit_pv_chunk(prev, ic, phase)
                        if phase == 3:
                            emit_norm_ic(prev, ic)
                            if ic == NCH - 1:
                                prev["ets"] = None
                    dance(n, t)

            # ---- tail: last outproj (ic3 of su3) ----
            su = NSU - 1
            emit_xt_pe(x_sbs[su], xts[su], 3)
            emit_outproj(xts[su], su, 3, os_on_act=True)

    nc.compile()
    return nc


def _prep_in_maps(query, key, value, mask, Wq, bq, Wk, bk, Wv, bv, Wo):
    in_maps = []
    for c in range(8):
        b = c // 2
        hh = c % 2
        ob = slice(hh * OB, (hh + 1) * OB)

        def xprep(x):
            xt = np.ascontiguousarray(np.asarray(x).T)     # [F, T]
            return np.ascontiguousarray(
                xt.reshape(FO, P, NC4, CW).transpose(2, 1, 0, 3)
            ).astype(np.float16)

        def wprep(W):
            wt = np.ascontiguousarray(np.asarray(W)[ob, :].T)   # [F, OB]
            return np.ascontiguousarray(
                wt.reshape(FO, P, OB).transpose(1, 0, 2)).astype(np.float16)

        woTh = np.ascontiguousarray(np.asarray(Wo)[:, ob].T)    # [OB, F]
        woTh = np.ascontiguousarray(
            woTh.reshape(2, P, F).transpose(1, 0, 2)).astype(np.float16)

        emb = (np.asarray(mask)[b, 0, :] != 0).astype(np.float32)
        emb = np.ascontiguousarray(emb.reshape(NT, P).T)

        in_maps.append({
            "xqT": xprep(query[b]),
            "xkT": xprep(key[b]),
            "xvT": xprep(value[b]),
            "wqT": wprep(Wq),
            "wkT": wprep(Wk),
            "wvT": wprep(Wv),
            "woT": woTh,
            "bqr": np.ascontiguousarray(bq[ob].reshape(OB // P, P).T),
            "bkr": np.ascontiguousarray(bk[ob].reshape(OB // P, P).T),
            "bvb": np.ascontiguousarray(np.tile(bv[ob][None, :], (P, 1))),
            "expmb": emb,
            "ident": np.eye(P, dtype=np.float16),
        })
    return in_maps


def kernel(query, key, value, mask, Wq, bq, Wk, bk, Wv, bv, Wo, bo):
    query = np.asarray(query, dtype=np.float32)
    key = np.asarray(key, dtype=np.float32)
    value = np.asarray(value, dtype=np.float32)
    mask = np.asarray(mask)
    Wq = np.asarray(Wq, dtype=np.float32)
    bq = np.asarray(bq, dtype=np.float32)
    Wk = np.asarray(Wk, dtype=np.float32)
    bk = np.asarray(bk, dtype=np.float32)
    Wv = np.asarray(Wv, dtype=np.float32)
    bv = np.asarray(bv, dtype=np.float32)
    Wo = np.asarray(Wo, dtype=np.float32)
    bo = np.asarray(bo, dtype=np.float32)

    if "nc" not in _CACHE:
        _CACHE["nc"] = _build()
    nc = _CACHE["nc"]

    B = query.shape[0]
    in_maps = _prep_in_maps(query, key, value, mask, Wq, bq, Wk, bk, Wv, bv, Wo)
    res = run_bass_kernel_spmd(nc, in_maps, core_ids=list(range(8)))

    outv = np.empty((B, T, F), dtype=np.float32)
    for b in range(B):
        outv[b] = (res.results[2 * b]["out"].astype(np.float32)
                   + res.results[2 * b + 1]["out"].astype(np.float32)
                   + bo[None, :])
    return outv
